# revision 36
# baseline (speedup 1.0000x reference)
"""Causal self-attention on 8 NeuronCores (TRN2), tensor-parallel over heads.

Reference: y = proj(softmax(causal(Q K^T / sqrt(64))) V) with
B=4, T=2048, D=1024, H=16 heads, head_dim=64.

Sharding: each core owns 2 heads (a 128-column slice of the Q/K/V
projections and the matching 128 rows of w_proj) for all batches. Each
core emits a partial [B*T, D] output; the host sums the 8 partials
(row-parallel matmul unshard) and reshapes to [B, T, D].
"""

import sys

for _p in ("/opt/trn_rl_repo",):
    if _p not in sys.path:
        sys.path.insert(0, _p)

import numpy as np

import concourse.bass as bass
import concourse.bacc as bacc
import concourse.mybir as mybir
from concourse import tile
from concourse.bass_utils import run_bass_kernel_spmd
from concourse.masks import make_identity

B, T, D, H = 4, 2048, 1024, 16
HD = D // H           # 64 head dim
NCORES = 8
HPC = H // NCORES     # 2 heads per core
CW = HPC * HD         # 128: per-core qkv column slice width
BT = B * T            # 8192 tokens
KC = D // 128         # 8 contraction chunks for the qkv projection
NQ = 512              # query chunk
NG = NQ // 128        # 4 key-tiles per S^T group
F32 = mybir.dt.float32
F32R = mybir.dt.float32r
BF16 = mybir.dt.bfloat16
EXP = mybir.ActivationFunctionType.Exp

VST = HPC * (HD + 1)  # 130: V tile stride (per head: 64 cols + ones col)


def build_kernel():
    nc = bacc.Bacc("TRN2", target_bir_lowering=False, debug=False)

    xT = nc.dram_tensor("xT", [D, BT], F32R, kind="ExternalInput")
    # wqkv packed on host as [128, KC, 3*CW]: (kc,:) = rows kc*128..+128 of
    # [w_q_slice | w_k_slice | w_v_slice]
    wqkv = nc.dram_tensor("wqkv", [128, KC * 3 * CW], F32R, kind="ExternalInput")
    wp = nc.dram_tensor("wp", [CW, D], F32R, kind="ExternalInput")
    out = nc.dram_tensor("out", [BT, D], F32, kind="ExternalOutput")

    with tile.TileContext(nc) as tc:
        _body(tc, xT.ap(), wqkv.ap(), wp.ap(), out.ap())
    nc.compile()
    return nc


def _body(tc, xT, wqkv, wp, out):
    nc = tc.nc
    with (
        tc.tile_pool(name="const", bufs=1) as const,
        tc.tile_pool(name="xin", bufs=3) as xin,
        tc.tile_pool(name="qk", bufs=2) as qkpool,
        tc.tile_pool(name="vb", bufs=2) as vbpool,
        tc.tile_pool(name="vs", bufs=2) as vspool,
        tc.tile_pool(name="pt", bufs=28) as ptpool,
        tc.tile_pool(name="ptd", bufs=2) as ptdpool,
        tc.tile_pool(name="yt", bufs=2) as ytpool,
        tc.tile_pool(name="dn", bufs=2) as dnpool,
        tc.tile_pool(name="os", bufs=3) as ospool,
        tc.tile_pool(name="pst", bufs=2, space="PSUM") as pst,
        tc.tile_pool(name="pav", bufs=1, space="PSUM") as pav,
        tc.tile_pool(name="psm", bufs=2, space="PSUM") as psm,
    ):
        # ---- constants ----
        wq_sb = const.tile([128, KC, 3 * CW], F32R, tag="wqkv")
        nc.sync.dma_start(wq_sb[:], wqkv.rearrange("p (k c) -> p k c", k=KC))
        wp_sb = const.tile([CW, D], F32R, tag="wp")
        nc.sync.dma_start(wp_sb[:], wp[:])
        ident = const.tile([128, 128], F32, tag="ident")
        make_identity(nc, ident[:])
        ones32 = const.tile([128, (T // 128) * HPC], BF16, tag="ones32")
        nc.gpsimd.memset(ones32[:], 1.0)
        scale = 1.0 / float(np.sqrt(HD))

        def make_qkv_steps(b, qt, kt, vb):
            """Emission steps for qkv_proj(b); each step is ~1.7us of PE
            work so it can be pulled one-at-a-time as filler inside another
            batch's attention kk-stream."""
            tok0 = b * T
            xts = {}

            def load(ch):
                xt = xin.tile([128, KC, NQ], F32R, tag="xt")
                for kc in range(KC):
                    nc.sync.dma_start(
                        xt[:, kc, :],
                        xT[kc * 128 : (kc + 1) * 128,
                           tok0 + ch * NQ : tok0 + (ch + 1) * NQ],
                    )
                xts[ch] = xt

            def mtile(ch, m, dst):
                if m == 0 and ch + 1 < T // NQ:
                    load(ch + 1)
                ps = psm.tile([128, NQ], F32, tag="ps")
                for kc in range(KC):
                    nc.tensor.matmul(
                        ps[:],
                        wq_sb[:, kc, m * CW : (m + 1) * CW],
                        xts[ch][:, kc, :],
                        start=(kc == 0),
                        stop=(kc == KC - 1),
                    )
                nc.vector.tensor_copy(dst[:, ch * NQ : (ch + 1) * NQ], ps[:])

            vss = {}

            def vtile_a(ch):
                ps = psm.tile([128, NQ], F32, tag="ps")
                for kc in range(KC):
                    nc.tensor.matmul(
                        ps[:],
                        wq_sb[:, kc, 2 * CW : 3 * CW],
                        xts[ch][:, kc, :],
                        start=(kc == 0),
                        stop=(kc == KC - 1),
                    )
                vs = vspool.tile([128, NQ], F32, tag="vs")
                nc.vector.tensor_copy(vs[:], ps[:])
                vss[ch] = vs

            def vtile_b(ch):
                vs = vss[ch]
                pt2 = psm.tile([128, NQ], F32, tag="ps")
                for q in range(NG):
                    nc.tensor.transpose(
                        pt2[:, q * 128 : (q + 1) * 128],
                        vs[:, q * 128 : (q + 1) * 128],
                        ident[:],
                    )
                # pt2 holds [tok 128][tile q: h0 64 | h1 64]; scatter into vb
                # (col 0 of each 65-col head block is the ones column)
                dstv = bass.AP(
                    vb.tensor,
                    vb[:].offset + ch * NG * VST,
                    [vb[:].ap[0], [VST, NG], [HD + 1, HPC], [1, HD]],
                )
                srcv = pt2[:].rearrange("p (t h d) -> p t h d", t=NG, h=HPC)
                nc.vector.tensor_copy(dstv, srcv)

            load(0)
            # ones columns (denominator trick): col 65*j + HD of vb; written
            # up front so early AV groups never wait on the last chunk
            onesv = bass.AP(
                vb.tensor,
                vb[:].offset + HD,
                [vb[:].ap[0], [HD + 1, (T // 128) * HPC]],
            )
            nc.vector.tensor_copy(onesv, ones32[:])
            steps = []
            from functools import partial
            for ch in range(T // NQ):
                steps.append(partial(mtile, ch, 0, qt))
                steps.append(partial(mtile, ch, 1, kt))
                steps.append(partial(vtile_a, ch))
                steps.append(partial(vtile_b, ch))
            return steps

        def finalize_norm(yt, jq, ytu):
            # divide O^T rows by the denominator row (broadcast to 64 parts);
            # single-op approx reciprocal (~18 bits) instead of the ~6-op
            # Newton lowering of reciprocal(). The custom-DVE recip and the
            # broadcast are partition-0-strict, so first move the denominator
            # row p64 -> p0 with a native copy.
            q0 = jq * NQ
            dn = dnpool.tile([1, HPC * NQ], F32, tag="dn")
            nc.vector.tensor_copy(dn[:], ytu[HD : HD + 1, :])
            nc.vector.reciprocal_approx_fast(dn[:], dn[:])
            dnb = dnpool.tile([HD, HPC * NQ], F32, tag="dnb")
            nc.gpsimd.partition_broadcast(dnb[:], dn[:])
            for h in range(HPC):
                nc.vector.tensor_mul(
                    yt[h * HD : (h + 1) * HD, q0 : q0 + NQ],
                    ytu[0:HD, h * NQ : (h + 1) * NQ],
                    dnb[:, h * NQ : (h + 1) * NQ],
                )

        def out_step(b, yt, tt):
            tok0 = b * T
            os_ = ospool.tile([128, D], F32, tag="os")
            for nn in range(D // NQ):
                pp = psm.tile([128, NQ], F32, tag="ps")
                nc.tensor.matmul(
                    pp[:],
                    yt[:, tt * 128 : (tt + 1) * 128],
                    wp_sb[:, nn * NQ : (nn + 1) * NQ],
                    start=True,
                    stop=True,
                )
                nc.vector.tensor_copy(os_[:, nn * NQ : (nn + 1) * NQ], pp[:])
            nc.sync.dma_start(
                out[tok0 + tt * 128 : tok0 + (tt + 1) * 128, :], os_[:]
            )

        def emit_av(avs, vb, nkk, kk, c0, ptk):
            for h in range(HPC):
                nc.tensor.matmul(
                    avs[h][0 : HD + 1, c0:NQ],
                    vb[:, kk * VST + h * (HD + 1) :
                         kk * VST + (h + 1) * (HD + 1)],
                    ptk[:, h * NQ + c0 : (h + 1) * NQ],
                    start=(kk == 0),
                    stop=(kk == nkk - 1),
                )

        def attention(b, qt, kt, vb, yt, fillers, carry, outq, finalized,
                      last=False):
            # AV matmuls run a FULL jq behind the S/exp pass: during jq's S
            # matmuls the PE drains the AVs of jq-1 (and, at jq=0, of the
            # previous batch's last jq), whose exp/affine outputs completed
            # long ago. The PE stream therefore never waits on the Act or
            # Pool engines, keeping the DVFS p-state pinned at max.
            def finish_carry():
                c = carry[0]
                if c is None or c["avq"]:
                    return
                # all AVs of the carried jq are emitted: evacuate + normalize
                avs_ = c["avs"]
                ytu = dnpool.tile([HD + 1, HPC * NQ], F32, tag="ytu")
                for h in range(HPC):
                    nc.vector.tensor_copy(
                        ytu[:, h * NQ : (h + 1) * NQ], avs_[h][0 : HD + 1, :]
                    )
                finalize_norm(c["yt"], c["jq"], ytu)
                # out tiles become eligible one jq LATER, giving the DVE
                # queue a full jq to actually execute the normalize before
                # the PE's out-proj matmul reads yt
                finalized.add((c["b"], c["jq"] - 1))
                if c["jq"] == 1:
                    finalized.add((c["b"] - 1, T // NQ - 1))
                carry[0] = None

            def emit_prev_avs(slots_left):
                c = carry[0]
                if c is None:
                    return
                n = (len(c["avq"]) + slots_left - 1) // max(slots_left, 1)
                for _ in range(n):
                    if not c["avq"]:
                        break
                    kk_, c0_, ptk_ = c["avq"].pop(0)
                    emit_av(c["avs"], c["vb"], c["nkk"], kk_, c0_, ptk_)
                finish_carry()

            def pull():
                # out-proj tiles whose normalize is emitted, else qkv steps.
                # The last batch holds back its final out tiles so drain_tail
                # has PE work to hide the last normalize chain behind.
                if (outq and (outq[0][0], outq[0][2] // NG) in finalized
                        and not (last and outq[0][2] >= 10)):
                    b_, yt_, tt_ = outq.pop(0)
                    out_step(b_, yt_, tt_)
                elif fillers:
                    fillers.pop(0)()

            for jq in range(T // NQ):
                q0 = jq * NQ
                nkk = NG * (jq + 1)
                diag0 = NG * jq
                av0 = pav.tile([128, NQ], F32, tag="av0")
                av1 = pav.tile([128, NQ], F32, tag="av1")
                cur = {"avs": [av0, av1], "nkk": nkk, "avq": [], "jq": jq,
                       "b": b, "vb": vb, "yt": yt}
                for kk in range(nkk):
                    i = kk - diag0          # >= 0 on the diagonal run
                    c0 = max(i, 0) * 128    # first valid q col in this chunk
                    w = NQ - c0
                    st = pst.tile([128, HPC * NQ], F32, tag="st")
                    for h in range(HPC):
                        nc.tensor.matmul(
                            st[:, h * NQ + c0 : (h + 1) * NQ],
                            kt[h * HD : (h + 1) * HD, kk * 128 : (kk + 1) * 128],
                            qt[h * HD : (h + 1) * HD, q0 + c0 : q0 + NQ],
                            start=True,
                            stop=True,
                        )
                    ptk = ptpool.tile([128, HPC * NQ], BF16, tag="pt")
                    stv = bass.AP(st.tensor, st[:].offset + c0,
                                  [st[:].ap[0], [NQ, HPC], [1, w]])
                    ptv = bass.AP(ptk.tensor, ptk[:].offset + c0,
                                  [ptk[:].ap[0], [NQ, HPC], [1, w]])
                    nc.scalar.activation(ptv, stv, EXP, scale=scale)
                    if i >= 0:
                        # zero q < kpart inside the 128-wide diagonal block
                        tri = bass.AP(ptk.tensor, ptk[:].offset + c0,
                                      [ptk[:].ap[0], [NQ, HPC], [1, 128]])
                        nc.gpsimd.affine_select(
                            out=tri,
                            in_=tri,
                            pattern=[[0, HPC], [1, 128]],
                            channel_multiplier=-1,
                            base=0,
                            compare_op=mybir.AluOpType.is_ge,
                            fill=0.0,
                        )
                    cur["avq"].append((kk, c0, ptk))
                    emit_prev_avs(nkk - kk)
                    if b == 0 and jq < 2:
                        pull()  # extra filler: densify the cold-start region
                    if last and carry[0] is None and len(cur["avq"]) > 1:
                        # tail batch: nothing left to interleave, so drain
                        # our own AVs one behind the S pass
                        kk_, c0_, ptk_ = cur["avq"].pop(0)
                        emit_av(cur["avs"], vb, nkk, kk_, c0_, ptk_)
                    pull()
                carry[0] = cur
            # leftover fillers (none in the steady state)
            while fillers:
                fillers.pop(0)()

        def drain_tail(carry, outq, finalized):
            # emit the final jq's AVs, its normalize, and remaining out tiles
            c = carry[0]
            if c is not None:
                while c["avq"]:
                    kk_, c0_, ptk_ = c["avq"].pop(0)
                    emit_av(c["avs"], c["vb"], c["nkk"], kk_, c0_, ptk_)
                ytu = dnpool.tile([HD + 1, HPC * NQ], F32, tag="ytu")
                for h in range(HPC):
                    nc.vector.tensor_copy(
                        ytu[:, h * NQ : (h + 1) * NQ], c["avs"][h][0 : HD + 1, :]
                    )
                finalize_norm(c["yt"], c["jq"], ytu)
                finalized.add((c["b"], c["jq"] - 1))
                finalized.add((c["b"], c["jq"]))
                carry[0] = None
            outq.sort(key=lambda e: (e[2] // NG != T // NQ - 1, e[2]))
            while outq:
                b_, yt_, tt_ = outq.pop(0)
                out_step(b_, yt_, tt_)

        tiles = []
        for b in range(B):
            qt = qkpool.tile([128, T], BF16, tag="qt")
            kt = qkpool.tile([128, T], BF16, tag="kt")
            vb = vbpool.tile([128, (T // 128) * VST], BF16, tag="vb")
            yt = ytpool.tile([128, T], F32R, tag="yt")
            tiles.append((qt, kt, vb, yt))

        # qkv(0) runs standalone; qkv(b+1) is pulled as filler during
        # attention(b); AVs lag a full jq behind their S/exp pass and cross
        # batch boundaries, so the PE pipeline never drains.
        carry = [None]
        finalized = set()
        outq = []
        steps0 = make_qkv_steps(0, *tiles[0][:3])
        for s in steps0[:4]:
            s()
        # chunks 1-3 of batch 0 ride the attention(0) filler stream: the
        # 1-filler-per-slot cadence emits chunk j just before jq=j needs it
        fillers = steps0[4:] + make_qkv_steps(1, *tiles[1][:3])
        for b in range(B):
            outq.extend((b, tiles[b][3], tt) for tt in range(T // 128))
            attention(b, *tiles[b], fillers, carry, outq, finalized,
                      last=(b == B - 1))
            fillers = (
                make_qkv_steps(b + 2, *tiles[b + 2][:3]) if b + 2 < B else []
            )
        drain_tail(carry, outq, finalized)


_NC_CACHE = None


def kernel(x: np.ndarray, w_attn: np.ndarray, w_proj: np.ndarray) -> np.ndarray:
    global _NC_CACHE
    if _NC_CACHE is None:
        _NC_CACHE = build_kernel()
    nc = _NC_CACHE

    x = np.asarray(x, dtype=np.float32)
    w_attn = np.asarray(w_attn, dtype=np.float32)
    w_proj = np.asarray(w_proj, dtype=np.float32)

    xT = np.ascontiguousarray(x.reshape(BT, D).T)  # [D, BT]

    in_maps = []
    for c in range(NCORES):
        c0 = c * CW
        wq = w_attn[:, c0 : c0 + CW]
        wk = w_attn[:, D + c0 : D + c0 + CW]
        wv = w_attn[:, 2 * D + c0 : 2 * D + c0 + CW]
        wslice = np.concatenate([wq, wk, wv], axis=1)          # [D, 3*CW]
        wpacked = np.ascontiguousarray(
            wslice.reshape(KC, 128, 3 * CW).transpose(1, 0, 2)
        ).reshape(128, KC * 3 * CW)
        wpc = np.ascontiguousarray(w_proj[c0 : c0 + CW, :])    # [CW, D]
        in_maps.append({"xT": xT, "wqkv": wpacked, "wp": wpc})

    res = run_bass_kernel_spmd(nc, in_maps, core_ids=list(range(NCORES)))
    acc = np.zeros((BT, D), dtype=np.float32)
    for r in res.results:
        acc += r["out"]
    return acc.reshape(B, T, D)


if __name__ == "__main__":
    inputs = {
        "x": np.random.randn(B, T, D).astype(np.float32),
        "w_attn": (np.random.randn(D, 3 * D) / np.sqrt(D)).astype(np.float32),
        "w_proj": (np.random.randn(D, D) / np.sqrt(D)).astype(np.float32),
    }
    y = kernel(**inputs)
    print(y.shape, y.dtype)



# revision 37
# speedup vs baseline: 1.0388x; 1.0388x over previous
"""Causal self-attention on 8 NeuronCores (TRN2), tensor-parallel over heads.

Reference: y = proj(softmax(causal(Q K^T / sqrt(64))) V) with
B=4, T=2048, D=1024, H=16 heads, head_dim=64.

Sharding: each core owns 2 heads (a 128-column slice of the Q/K/V
projections and the matching 128 rows of w_proj) for all batches. Each
core emits a partial [B*T, D] output; the host sums the 8 partials
(row-parallel matmul unshard) and reshapes to [B, T, D].
"""

import sys

for _p in ("/opt/trn_rl_repo",):
    if _p not in sys.path:
        sys.path.insert(0, _p)

import numpy as np

import concourse.bass as bass
import concourse.bacc as bacc
import concourse.mybir as mybir
from concourse import tile
from concourse.bass_utils import run_bass_kernel_spmd
from concourse.masks import make_identity

B, T, D, H = 4, 2048, 1024, 16
HD = D // H           # 64 head dim
NCORES = 8
HPC = H // NCORES     # 2 heads per core
CW = HPC * HD         # 128: per-core qkv column slice width
BT = B * T            # 8192 tokens
KC = D // 128         # 8 contraction chunks for the qkv projection
NQ = 512              # query chunk
NG = NQ // 128        # 4 key-tiles per S^T group
F32 = mybir.dt.float32
F32R = mybir.dt.float32r
BF16 = mybir.dt.bfloat16
EXP = mybir.ActivationFunctionType.Exp

VST = HPC * (HD + 1)  # 130: V tile stride (per head: 64 cols + ones col)


def build_kernel():
    nc = bacc.Bacc("TRN2", target_bir_lowering=False, debug=False)

    xT = nc.dram_tensor("xT", [D, BT], F32R, kind="ExternalInput")
    # wqkv packed on host as [128, KC, 3*CW]: (kc,:) = rows kc*128..+128 of
    # [w_q_slice | w_k_slice | w_v_slice]
    wqkv = nc.dram_tensor("wqkv", [128, KC * 3 * CW], F32R, kind="ExternalInput")
    wp = nc.dram_tensor("wp", [CW, D], F32R, kind="ExternalInput")
    out = nc.dram_tensor("out", [BT, D], F32, kind="ExternalOutput")

    with tile.TileContext(nc) as tc:
        _body(tc, xT.ap(), wqkv.ap(), wp.ap(), out.ap())
    nc.compile()
    return nc


def _body(tc, xT, wqkv, wp, out):
    nc = tc.nc
    with (
        tc.tile_pool(name="const", bufs=1) as const,
        tc.tile_pool(name="xin", bufs=3) as xin,
        tc.tile_pool(name="qk", bufs=2) as qkpool,
        tc.tile_pool(name="vb", bufs=2) as vbpool,
        tc.tile_pool(name="vs", bufs=2) as vspool,
        tc.tile_pool(name="pt", bufs=28) as ptpool,
        tc.tile_pool(name="ptd", bufs=2) as ptdpool,
        tc.tile_pool(name="yt", bufs=2) as ytpool,
        tc.tile_pool(name="dn", bufs=2) as dnpool,
        tc.tile_pool(name="os", bufs=3) as ospool,
        tc.tile_pool(name="pst", bufs=2, space="PSUM") as pst,
        tc.tile_pool(name="pav", bufs=1, space="PSUM") as pav,
        tc.tile_pool(name="psm", bufs=2, space="PSUM") as psm,
    ):
        # ---- constants ----
        wq_sb = const.tile([128, KC, 3 * CW], F32R, tag="wqkv")
        nc.sync.dma_start(wq_sb[:], wqkv.rearrange("p (k c) -> p k c", k=KC))
        wp_sb = const.tile([CW, D], F32R, tag="wp")
        nc.sync.dma_start(wp_sb[:], wp[:])
        ident = const.tile([128, 128], F32, tag="ident")
        make_identity(nc, ident[:])
        ones32 = const.tile([128, (T // 128) * HPC], BF16, tag="ones32")
        nc.gpsimd.memset(ones32[:], 1.0)
        scale = 1.0 / float(np.sqrt(HD))

        def make_qkv_steps(b, qt, kt, vb):
            """Emission steps for qkv_proj(b); each step is ~1.7us of PE
            work so it can be pulled one-at-a-time as filler inside another
            batch's attention kk-stream."""
            tok0 = b * T
            xts = {}

            def load(ch):
                xt = xin.tile([128, KC, NQ], F32R, tag="xt")
                for kc in range(KC):
                    nc.sync.dma_start(
                        xt[:, kc, :],
                        xT[kc * 128 : (kc + 1) * 128,
                           tok0 + ch * NQ : tok0 + (ch + 1) * NQ],
                    )
                xts[ch] = xt

            def mtile(ch, m, dst):
                if m == 0 and ch + 1 < T // NQ:
                    load(ch + 1)
                ps = psm.tile([128, NQ], F32, tag="ps")
                for kc in range(KC):
                    nc.tensor.matmul(
                        ps[:],
                        wq_sb[:, kc, m * CW : (m + 1) * CW],
                        xts[ch][:, kc, :],
                        start=(kc == 0),
                        stop=(kc == KC - 1),
                    )
                nc.vector.tensor_copy(dst[:, ch * NQ : (ch + 1) * NQ], ps[:])

            vss = {}

            def vtile_a(ch):
                ps = psm.tile([128, NQ], F32, tag="ps")
                for kc in range(KC):
                    nc.tensor.matmul(
                        ps[:],
                        wq_sb[:, kc, 2 * CW : 3 * CW],
                        xts[ch][:, kc, :],
                        start=(kc == 0),
                        stop=(kc == KC - 1),
                    )
                vs = vspool.tile([128, NQ], F32, tag="vs")
                nc.vector.tensor_copy(vs[:], ps[:])
                vss[ch] = vs

            def vtile_b(ch):
                vs = vss[ch]
                pt2 = psm.tile([128, NQ], F32, tag="ps")
                for q in range(NG):
                    nc.tensor.transpose(
                        pt2[:, q * 128 : (q + 1) * 128],
                        vs[:, q * 128 : (q + 1) * 128],
                        ident[:],
                    )
                # pt2 holds [tok 128][tile q: h0 64 | h1 64]; scatter into vb
                # (col 0 of each 65-col head block is the ones column)
                dstv = bass.AP(
                    vb.tensor,
                    vb[:].offset + ch * NG * VST,
                    [vb[:].ap[0], [VST, NG], [HD + 1, HPC], [1, HD]],
                )
                srcv = pt2[:].rearrange("p (t h d) -> p t h d", t=NG, h=HPC)
                nc.vector.tensor_copy(dstv, srcv)

            load(0)
            # ones columns (denominator trick): col 65*j + HD of vb; written
            # up front so early AV groups never wait on the last chunk
            onesv = bass.AP(
                vb.tensor,
                vb[:].offset + HD,
                [vb[:].ap[0], [HD + 1, (T // 128) * HPC]],
            )
            nc.vector.tensor_copy(onesv, ones32[:])
            steps = []
            from functools import partial
            for ch in range(T // NQ):
                steps.append(partial(mtile, ch, 0, qt))
                steps.append(partial(mtile, ch, 1, kt))
                steps.append(partial(vtile_a, ch))
                steps.append(partial(vtile_b, ch))
            return steps

        def finalize_norm(yt, jq, ytu):
            # divide O^T rows by the denominator row (broadcast to 64 parts);
            # single-op approx reciprocal (~18 bits) instead of the ~6-op
            # Newton lowering of reciprocal(). The custom-DVE recip and the
            # broadcast are partition-0-strict, so first move the denominator
            # row p64 -> p0 with a native copy.
            q0 = jq * NQ
            dn = dnpool.tile([1, HPC * NQ], F32, tag="dn")
            nc.vector.tensor_copy(dn[:], ytu[HD : HD + 1, :])
            nc.vector.reciprocal_approx_fast(dn[:], dn[:])
            dnb = dnpool.tile([HD, HPC * NQ], F32, tag="dnb")
            nc.gpsimd.partition_broadcast(dnb[:], dn[:])
            for h in range(HPC):
                nc.vector.tensor_mul(
                    yt[h * HD : (h + 1) * HD, q0 : q0 + NQ],
                    ytu[0:HD, h * NQ : (h + 1) * NQ],
                    dnb[:, h * NQ : (h + 1) * NQ],
                )

        def out_step(b, yt, tt):
            tok0 = b * T
            os_ = ospool.tile([128, D], F32, tag="os")
            for nn in range(D // NQ):
                pp = psm.tile([128, NQ], F32, tag="ps")
                nc.tensor.matmul(
                    pp[:],
                    yt[:, tt * 128 : (tt + 1) * 128],
                    wp_sb[:, nn * NQ : (nn + 1) * NQ],
                    start=True,
                    stop=True,
                )
                nc.vector.tensor_copy(os_[:, nn * NQ : (nn + 1) * NQ], pp[:])
            nc.sync.dma_start(
                out[tok0 + tt * 128 : tok0 + (tt + 1) * 128, :], os_[:]
            )

        def emit_av(avs, vb, nkk, kk, c0, ptk):
            for h in range(HPC):
                nc.tensor.matmul(
                    avs[h][0 : HD + 1, c0:NQ],
                    vb[:, kk * VST + h * (HD + 1) :
                         kk * VST + (h + 1) * (HD + 1)],
                    ptk[:, h * NQ + c0 : (h + 1) * NQ],
                    start=(kk == 0),
                    stop=(kk == nkk - 1),
                )

        def attention(b, qt, kt, vb, yt, fillers, carry, outq, finalized,
                      last=False):
            # AV matmuls run a FULL jq behind the S/exp pass: during jq's S
            # matmuls the PE drains the AVs of jq-1 (and, at jq=0, of the
            # previous batch's last jq), whose exp/affine outputs completed
            # long ago. The PE stream therefore never waits on the Act or
            # Pool engines, keeping the DVFS p-state pinned at max.
            def finish_carry():
                c = carry[0]
                if c is None or c["avq"]:
                    return
                # all AVs of the carried jq are emitted: evacuate + normalize
                avs_ = c["avs"]
                ytu = dnpool.tile([HD + 1, HPC * NQ], F32, tag="ytu")
                for h in range(HPC):
                    nc.vector.tensor_copy(
                        ytu[:, h * NQ : (h + 1) * NQ], avs_[h][0 : HD + 1, :]
                    )
                finalize_norm(c["yt"], c["jq"], ytu)
                # out tiles become eligible one jq LATER, giving the DVE
                # queue a full jq to actually execute the normalize before
                # the PE's out-proj matmul reads yt
                finalized.add((c["b"], c["jq"] - 1))
                if c["jq"] == 1:
                    finalized.add((c["b"] - 1, T // NQ - 1))
                carry[0] = None

            def emit_prev_avs(slots_left):
                c = carry[0]
                if c is None:
                    return
                n = (len(c["avq"]) + slots_left - 1) // max(slots_left, 1)
                for _ in range(n):
                    if not c["avq"]:
                        break
                    kk_, c0_, ptk_ = c["avq"].pop(0)
                    emit_av(c["avs"], c["vb"], c["nkk"], kk_, c0_, ptk_)
                finish_carry()

            def pull():
                # out-proj tiles whose normalize is emitted, else qkv steps.
                # The last batch holds back its final out tiles so drain_tail
                # has PE work to hide the last normalize chain behind.
                if (outq and (outq[0][0], outq[0][2] // NG) in finalized
                        and not (last and outq[0][2] >= 10)):
                    b_, yt_, tt_ = outq.pop(0)
                    out_step(b_, yt_, tt_)
                elif fillers:
                    fillers.pop(0)()

            for jq in range(T // NQ):
                q0 = jq * NQ
                nkk = NG * (jq + 1)
                diag0 = NG * jq
                av0 = pav.tile([128, NQ], F32, tag="av0")
                av1 = pav.tile([128, NQ], F32, tag="av1")
                cur = {"avs": [av0, av1], "nkk": nkk, "avq": [], "jq": jq,
                       "b": b, "vb": vb, "yt": yt}
                for kk in range(nkk):
                    i = kk - diag0          # >= 0 on the diagonal run
                    c0 = max(i, 0) * 128    # first valid q col in this chunk
                    w = NQ - c0
                    st = pst.tile([128, HPC * NQ], F32, tag="st")
                    for h in range(HPC):
                        nc.tensor.matmul(
                            st[:, h * NQ + c0 : (h + 1) * NQ],
                            kt[h * HD : (h + 1) * HD, kk * 128 : (kk + 1) * 128],
                            qt[h * HD : (h + 1) * HD, q0 + c0 : q0 + NQ],
                            start=True,
                            stop=True,
                        )
                    ptk = ptpool.tile([128, HPC * NQ], BF16, tag="pt")
                    stv = bass.AP(st.tensor, st[:].offset + c0,
                                  [st[:].ap[0], [NQ, HPC], [1, w]])
                    ptv = bass.AP(ptk.tensor, ptk[:].offset + c0,
                                  [ptk[:].ap[0], [NQ, HPC], [1, w]])
                    nc.scalar.activation(ptv, stv, EXP, scale=scale)
                    if i >= 0:
                        # zero q < kpart inside the 128-wide diagonal block
                        tri = bass.AP(ptk.tensor, ptk[:].offset + c0,
                                      [ptk[:].ap[0], [NQ, HPC], [1, 128]])
                        nc.gpsimd.affine_select(
                            out=tri,
                            in_=tri,
                            pattern=[[0, HPC], [1, 128]],
                            channel_multiplier=-1,
                            base=0,
                            compare_op=mybir.AluOpType.is_ge,
                            fill=0.0,
                        )
                    cur["avq"].append((kk, c0, ptk))
                    emit_prev_avs(nkk - kk)
                    if b == 0 and jq < 2:
                        pull()  # extra filler: densify the cold-start region
                    if last and carry[0] is None and len(cur["avq"]) > 1:
                        # tail batch: nothing left to interleave, so drain
                        # our own AVs one behind the S pass
                        kk_, c0_, ptk_ = cur["avq"].pop(0)
                        emit_av(cur["avs"], vb, nkk, kk_, c0_, ptk_)
                    pull()
                carry[0] = cur
            # leftover fillers (none in the steady state)
            while fillers:
                fillers.pop(0)()

        def drain_tail(carry, outq, finalized):
            # emit the final jq's AVs, its normalize, and remaining out tiles
            c = carry[0]
            if c is not None:
                while c["avq"]:
                    kk_, c0_, ptk_ = c["avq"].pop(0)
                    emit_av(c["avs"], c["vb"], c["nkk"], kk_, c0_, ptk_)
                ytu = dnpool.tile([HD + 1, HPC * NQ], F32, tag="ytu")
                for h in range(HPC):
                    nc.vector.tensor_copy(
                        ytu[:, h * NQ : (h + 1) * NQ], c["avs"][h][0 : HD + 1, :]
                    )
                finalize_norm(c["yt"], c["jq"], ytu)
                finalized.add((c["b"], c["jq"] - 1))
                finalized.add((c["b"], c["jq"]))
                carry[0] = None
            outq.sort(key=lambda e: (e[2] // NG == T // NQ - 1, e[2]))
            while outq:
                b_, yt_, tt_ = outq.pop(0)
                out_step(b_, yt_, tt_)

        tiles = []
        for b in range(B):
            qt = qkpool.tile([128, T], BF16, tag="qt")
            kt = qkpool.tile([128, T], BF16, tag="kt")
            vb = vbpool.tile([128, (T // 128) * VST], BF16, tag="vb")
            yt = ytpool.tile([128, T], F32R, tag="yt")
            tiles.append((qt, kt, vb, yt))

        # qkv(0) runs standalone; qkv(b+1) is pulled as filler during
        # attention(b); AVs lag a full jq behind their S/exp pass and cross
        # batch boundaries, so the PE pipeline never drains.
        carry = [None]
        finalized = set()
        outq = []
        steps0 = make_qkv_steps(0, *tiles[0][:3])
        for s in steps0[:4]:
            s()
        # chunks 1-3 of batch 0 ride the attention(0) filler stream: the
        # 1-filler-per-slot cadence emits chunk j just before jq=j needs it
        fillers = steps0[4:] + make_qkv_steps(1, *tiles[1][:3])
        for b in range(B):
            outq.extend((b, tiles[b][3], tt) for tt in range(T // 128))
            attention(b, *tiles[b], fillers, carry, outq, finalized,
                      last=(b == B - 1))
            fillers = (
                make_qkv_steps(b + 2, *tiles[b + 2][:3]) if b + 2 < B else []
            )
        drain_tail(carry, outq, finalized)


_NC_CACHE = None


def kernel(x: np.ndarray, w_attn: np.ndarray, w_proj: np.ndarray) -> np.ndarray:
    global _NC_CACHE
    if _NC_CACHE is None:
        _NC_CACHE = build_kernel()
    nc = _NC_CACHE

    x = np.asarray(x, dtype=np.float32)
    w_attn = np.asarray(w_attn, dtype=np.float32)
    w_proj = np.asarray(w_proj, dtype=np.float32)

    xT = np.ascontiguousarray(x.reshape(BT, D).T)  # [D, BT]

    in_maps = []
    for c in range(NCORES):
        c0 = c * CW
        wq = w_attn[:, c0 : c0 + CW]
        wk = w_attn[:, D + c0 : D + c0 + CW]
        wv = w_attn[:, 2 * D + c0 : 2 * D + c0 + CW]
        wslice = np.concatenate([wq, wk, wv], axis=1)          # [D, 3*CW]
        wpacked = np.ascontiguousarray(
            wslice.reshape(KC, 128, 3 * CW).transpose(1, 0, 2)
        ).reshape(128, KC * 3 * CW)
        wpc = np.ascontiguousarray(w_proj[c0 : c0 + CW, :])    # [CW, D]
        in_maps.append({"xT": xT, "wqkv": wpacked, "wp": wpc})

    res = run_bass_kernel_spmd(nc, in_maps, core_ids=list(range(NCORES)))
    acc = np.zeros((BT, D), dtype=np.float32)
    for r in res.results:
        acc += r["out"]
    return acc.reshape(B, T, D)


if __name__ == "__main__":
    inputs = {
        "x": np.random.randn(B, T, D).astype(np.float32),
        "w_attn": (np.random.randn(D, 3 * D) / np.sqrt(D)).astype(np.float32),
        "w_proj": (np.random.randn(D, D) / np.sqrt(D)).astype(np.float32),
    }
    y = kernel(**inputs)
    print(y.shape, y.dtype)



# revision 38
# speedup vs baseline: 1.0528x; 1.0136x over previous
"""Causal self-attention on 8 NeuronCores (TRN2), tensor-parallel over heads.

Reference: y = proj(softmax(causal(Q K^T / sqrt(64))) V) with
B=4, T=2048, D=1024, H=16 heads, head_dim=64.

Sharding: each core owns 2 heads (a 128-column slice of the Q/K/V
projections and the matching 128 rows of w_proj) for all batches. Each
core emits a partial [B*T, D] output; the host sums the 8 partials
(row-parallel matmul unshard) and reshapes to [B, T, D].
"""

import sys

for _p in ("/opt/trn_rl_repo",):
    if _p not in sys.path:
        sys.path.insert(0, _p)

import numpy as np

import concourse.bass as bass
import concourse.bacc as bacc
import concourse.mybir as mybir
from concourse import tile
from concourse.bass_utils import run_bass_kernel_spmd
from concourse.masks import make_identity

B, T, D, H = 4, 2048, 1024, 16
HD = D // H           # 64 head dim
NCORES = 8
HPC = H // NCORES     # 2 heads per core
CW = HPC * HD         # 128: per-core qkv column slice width
BT = B * T            # 8192 tokens
KC = D // 128         # 8 contraction chunks for the qkv projection
NQ = 512              # query chunk
NG = NQ // 128        # 4 key-tiles per S^T group
F32 = mybir.dt.float32
F32R = mybir.dt.float32r
BF16 = mybir.dt.bfloat16
EXP = mybir.ActivationFunctionType.Exp

VST = HPC * (HD + 1)  # 130: V tile stride (per head: 64 cols + ones col)


def build_kernel():
    nc = bacc.Bacc("TRN2", target_bir_lowering=False, debug=False)

    xT = nc.dram_tensor("xT", [D, BT], F32R, kind="ExternalInput")
    # wqkv packed on host as [128, KC, 3*CW]: (kc,:) = rows kc*128..+128 of
    # [w_q_slice | w_k_slice | w_v_slice]
    wqkv = nc.dram_tensor("wqkv", [128, KC * 3 * CW], F32R, kind="ExternalInput")
    wp = nc.dram_tensor("wp", [CW, D], F32R, kind="ExternalInput")
    out = nc.dram_tensor("out", [BT, D], F32, kind="ExternalOutput")

    with tile.TileContext(nc) as tc:
        _body(tc, xT.ap(), wqkv.ap(), wp.ap(), out.ap())
    nc.compile()
    return nc


def _body(tc, xT, wqkv, wp, out):
    nc = tc.nc
    with (
        tc.tile_pool(name="const", bufs=1) as const,
        tc.tile_pool(name="xin", bufs=3) as xin,
        tc.tile_pool(name="qk", bufs=2) as qkpool,
        tc.tile_pool(name="vb", bufs=2) as vbpool,
        tc.tile_pool(name="vs", bufs=2) as vspool,
        tc.tile_pool(name="pt", bufs=28) as ptpool,
        tc.tile_pool(name="ptd", bufs=2) as ptdpool,
        tc.tile_pool(name="yt", bufs=2) as ytpool,
        tc.tile_pool(name="dn", bufs=2) as dnpool,
        tc.tile_pool(name="os", bufs=3) as ospool,
        tc.tile_pool(name="pst", bufs=2, space="PSUM") as pst,
        tc.tile_pool(name="pav", bufs=1, space="PSUM") as pav,
        tc.tile_pool(name="psm", bufs=2, space="PSUM") as psm,
    ):
        # ---- constants ----
        wq_sb = const.tile([128, KC, 3 * CW], F32R, tag="wqkv")
        nc.sync.dma_start(wq_sb[:], wqkv.rearrange("p (k c) -> p k c", k=KC))
        wp_sb = const.tile([CW, D], F32R, tag="wp")
        nc.sync.dma_start(wp_sb[:], wp[:])
        ident = const.tile([128, 128], F32, tag="ident")
        make_identity(nc, ident[:])
        ones32 = const.tile([128, (T // 128) * HPC], BF16, tag="ones32")
        nc.gpsimd.memset(ones32[:], 1.0)
        scale = 1.0 / float(np.sqrt(HD))

        def make_qkv_steps(b, qt, kt, vb):
            """Emission steps for qkv_proj(b); each step is ~1.7us of PE
            work so it can be pulled one-at-a-time as filler inside another
            batch's attention kk-stream."""
            tok0 = b * T
            xts = {}

            def load(ch):
                xt = xin.tile([128, KC, NQ], F32R, tag="xt")
                for kc in range(KC):
                    nc.sync.dma_start(
                        xt[:, kc, :],
                        xT[kc * 128 : (kc + 1) * 128,
                           tok0 + ch * NQ : tok0 + (ch + 1) * NQ],
                    )
                xts[ch] = xt

            def mtile(ch, m, dst):
                if m == 0 and ch + 1 < T // NQ:
                    load(ch + 1)
                ps = psm.tile([128, NQ], F32, tag="ps")
                for kc in range(KC):
                    nc.tensor.matmul(
                        ps[:],
                        wq_sb[:, kc, m * CW : (m + 1) * CW],
                        xts[ch][:, kc, :],
                        start=(kc == 0),
                        stop=(kc == KC - 1),
                    )
                nc.vector.tensor_copy(dst[:, ch * NQ : (ch + 1) * NQ], ps[:])

            vss = {}

            def vtile_a(ch):
                ps = psm.tile([128, NQ], F32, tag="ps")
                for kc in range(KC):
                    nc.tensor.matmul(
                        ps[:],
                        wq_sb[:, kc, 2 * CW : 3 * CW],
                        xts[ch][:, kc, :],
                        start=(kc == 0),
                        stop=(kc == KC - 1),
                    )
                vs = vspool.tile([128, NQ], F32, tag="vs")
                nc.vector.tensor_copy(vs[:], ps[:])
                vss[ch] = vs

            def vtile_b(ch):
                vs = vss[ch]
                pt2 = psm.tile([128, NQ], F32, tag="ps")
                for q in range(NG):
                    nc.tensor.transpose(
                        pt2[:, q * 128 : (q + 1) * 128],
                        vs[:, q * 128 : (q + 1) * 128],
                        ident[:],
                    )
                # pt2 holds [tok 128][tile q: h0 64 | h1 64]; scatter into vb
                # (col 0 of each 65-col head block is the ones column)
                dstv = bass.AP(
                    vb.tensor,
                    vb[:].offset + ch * NG * VST,
                    [vb[:].ap[0], [VST, NG], [HD + 1, HPC], [1, HD]],
                )
                srcv = pt2[:].rearrange("p (t h d) -> p t h d", t=NG, h=HPC)
                nc.vector.tensor_copy(dstv, srcv)

            load(0)
            # ones columns (denominator trick): col 65*j + HD of vb; written
            # up front so early AV groups never wait on the last chunk
            onesv = bass.AP(
                vb.tensor,
                vb[:].offset + HD,
                [vb[:].ap[0], [HD + 1, (T // 128) * HPC]],
            )
            nc.vector.tensor_copy(onesv, ones32[:])
            steps = []
            from functools import partial
            for ch in range(T // NQ):
                steps.append(partial(mtile, ch, 0, qt))
                steps.append(partial(mtile, ch, 1, kt))
                steps.append(partial(vtile_a, ch))
                steps.append(partial(vtile_b, ch))
            return steps

        def finalize_norm(yt, jq, ytu):
            # divide O^T rows by the denominator row (broadcast to 64 parts);
            # single-op approx reciprocal (~18 bits) instead of the ~6-op
            # Newton lowering of reciprocal(). The custom-DVE recip and the
            # broadcast are partition-0-strict, so first move the denominator
            # row p64 -> p0 with a native copy.
            q0 = jq * NQ
            dn = dnpool.tile([1, HPC * NQ], F32, tag="dn")
            nc.vector.tensor_copy(dn[:], ytu[HD : HD + 1, :])
            nc.vector.reciprocal_approx_fast(dn[:], dn[:])
            dnb = dnpool.tile([HD, HPC * NQ], F32, tag="dnb")
            nc.gpsimd.partition_broadcast(dnb[:], dn[:])
            for h in range(HPC):
                nc.vector.tensor_mul(
                    yt[h * HD : (h + 1) * HD, q0 : q0 + NQ],
                    ytu[0:HD, h * NQ : (h + 1) * NQ],
                    dnb[:, h * NQ : (h + 1) * NQ],
                )

        def out_step(b, yt, tt):
            tok0 = b * T
            os_ = ospool.tile([128, D], F32, tag="os")
            for nn in range(D // NQ):
                pp = psm.tile([128, NQ], F32, tag="ps")
                nc.tensor.matmul(
                    pp[:],
                    yt[:, tt * 128 : (tt + 1) * 128],
                    wp_sb[:, nn * NQ : (nn + 1) * NQ],
                    start=True,
                    stop=True,
                )
                nc.vector.tensor_copy(os_[:, nn * NQ : (nn + 1) * NQ], pp[:])
            nc.sync.dma_start(
                out[tok0 + tt * 128 : tok0 + (tt + 1) * 128, :], os_[:]
            )

        def emit_av(avs, vb, nkk, kk, c0, ptk):
            for h in range(HPC):
                nc.tensor.matmul(
                    avs[h][0 : HD + 1, c0:NQ],
                    vb[:, kk * VST + h * (HD + 1) :
                         kk * VST + (h + 1) * (HD + 1)],
                    ptk[:, h * NQ + c0 : (h + 1) * NQ],
                    start=(kk == 0),
                    stop=(kk == nkk - 1),
                )

        def attention(b, qt, kt, vb, yt, fillers, carry, outq, finalized,
                      last=False):
            # AV matmuls run a FULL jq behind the S/exp pass: during jq's S
            # matmuls the PE drains the AVs of jq-1 (and, at jq=0, of the
            # previous batch's last jq), whose exp/affine outputs completed
            # long ago. The PE stream therefore never waits on the Act or
            # Pool engines, keeping the DVFS p-state pinned at max.
            def finish_carry():
                c = carry[0]
                if c is None or c["avq"]:
                    return
                # all AVs of the carried jq are emitted: evacuate + normalize
                avs_ = c["avs"]
                ytu = dnpool.tile([HD + 1, HPC * NQ], F32, tag="ytu")
                for h in range(HPC):
                    nc.vector.tensor_copy(
                        ytu[:, h * NQ : (h + 1) * NQ], avs_[h][0 : HD + 1, :]
                    )
                finalize_norm(c["yt"], c["jq"], ytu)
                # out tiles become eligible one jq LATER, giving the DVE
                # queue a full jq to actually execute the normalize before
                # the PE's out-proj matmul reads yt
                finalized.add((c["b"], c["jq"] - 1))
                if c["jq"] == 1:
                    finalized.add((c["b"] - 1, T // NQ - 1))
                carry[0] = None

            def emit_prev_avs(slots_left):
                c = carry[0]
                if c is None:
                    return
                n = (len(c["avq"]) + slots_left - 1) // max(slots_left, 1)
                for _ in range(n):
                    if not c["avq"]:
                        break
                    kk_, c0_, ptk_ = c["avq"].pop(0)
                    emit_av(c["avs"], c["vb"], c["nkk"], kk_, c0_, ptk_)
                finish_carry()

            def pull():
                # out-proj tiles whose normalize is emitted, else qkv steps
                if outq and (outq[0][0], outq[0][2] // NG) in finalized:
                    b_, yt_, tt_ = outq.pop(0)
                    out_step(b_, yt_, tt_)
                elif fillers:
                    fillers.pop(0)()

            for jq in range(T // NQ):
                q0 = jq * NQ
                nkk = NG * (jq + 1)
                diag0 = NG * jq
                av0 = pav.tile([128, NQ], F32, tag="av0")
                av1 = pav.tile([128, NQ], F32, tag="av1")
                cur = {"avs": [av0, av1], "nkk": nkk, "avq": [], "jq": jq,
                       "b": b, "vb": vb, "yt": yt}
                for kk in range(nkk):
                    i = kk - diag0          # >= 0 on the diagonal run
                    c0 = max(i, 0) * 128    # first valid q col in this chunk
                    w = NQ - c0
                    st = pst.tile([128, HPC * NQ], F32, tag="st")
                    for h in range(HPC):
                        nc.tensor.matmul(
                            st[:, h * NQ + c0 : (h + 1) * NQ],
                            kt[h * HD : (h + 1) * HD, kk * 128 : (kk + 1) * 128],
                            qt[h * HD : (h + 1) * HD, q0 + c0 : q0 + NQ],
                            start=True,
                            stop=True,
                        )
                    ptk = ptpool.tile([128, HPC * NQ], BF16, tag="pt")
                    stv = bass.AP(st.tensor, st[:].offset + c0,
                                  [st[:].ap[0], [NQ, HPC], [1, w]])
                    ptv = bass.AP(ptk.tensor, ptk[:].offset + c0,
                                  [ptk[:].ap[0], [NQ, HPC], [1, w]])
                    nc.scalar.activation(ptv, stv, EXP, scale=scale)
                    if i >= 0:
                        # zero q < kpart inside the 128-wide diagonal block
                        tri = bass.AP(ptk.tensor, ptk[:].offset + c0,
                                      [ptk[:].ap[0], [NQ, HPC], [1, 128]])
                        nc.gpsimd.affine_select(
                            out=tri,
                            in_=tri,
                            pattern=[[0, HPC], [1, 128]],
                            channel_multiplier=-1,
                            base=0,
                            compare_op=mybir.AluOpType.is_ge,
                            fill=0.0,
                        )
                    cur["avq"].append((kk, c0, ptk))
                    emit_prev_avs(nkk - kk)
                    if last and carry[0] is None and len(cur["avq"]) > 1:
                        # tail batch: nothing left to interleave, so drain
                        # our own AVs one behind the S pass
                        kk_, c0_, ptk_ = cur["avq"].pop(0)
                        emit_av(cur["avs"], vb, nkk, kk_, c0_, ptk_)
                    pull()
                carry[0] = cur
            # leftover fillers (none in the steady state)
            while fillers:
                fillers.pop(0)()

        def drain_tail(carry, outq, finalized):
            # emit the final jq's AVs, its normalize, and remaining out tiles
            c = carry[0]
            if c is not None:
                while c["avq"]:
                    kk_, c0_, ptk_ = c["avq"].pop(0)
                    emit_av(c["avs"], c["vb"], c["nkk"], kk_, c0_, ptk_)
                ytu = dnpool.tile([HD + 1, HPC * NQ], F32, tag="ytu")
                for h in range(HPC):
                    nc.vector.tensor_copy(
                        ytu[:, h * NQ : (h + 1) * NQ], c["avs"][h][0 : HD + 1, :]
                    )
                finalize_norm(c["yt"], c["jq"], ytu)
                finalized.add((c["b"], c["jq"] - 1))
                finalized.add((c["b"], c["jq"]))
                carry[0] = None
            while outq:
                b_, yt_, tt_ = outq.pop(0)
                out_step(b_, yt_, tt_)

        tiles = []
        for b in range(B):
            qt = qkpool.tile([128, T], BF16, tag="qt")
            kt = qkpool.tile([128, T], BF16, tag="kt")
            vb = vbpool.tile([128, (T // 128) * VST], BF16, tag="vb")
            yt = ytpool.tile([128, T], F32R, tag="yt")
            tiles.append((qt, kt, vb, yt))

        # qkv(0) runs standalone; qkv(b+1) is pulled as filler during
        # attention(b); AVs lag a full jq behind their S/exp pass and cross
        # batch boundaries, so the PE pipeline never drains.
        carry = [None]
        finalized = set()
        outq = []
        steps0 = make_qkv_steps(0, *tiles[0][:3])
        for s in steps0[:4]:
            s()
        # chunks 1-3 of batch 0 ride the attention(0) filler stream: the
        # 1-filler-per-slot cadence emits chunk j just before jq=j needs it
        fillers = steps0[4:] + make_qkv_steps(1, *tiles[1][:3])
        for b in range(B):
            outq.extend((b, tiles[b][3], tt) for tt in range(T // 128))
            attention(b, *tiles[b], fillers, carry, outq, finalized,
                      last=(b == B - 1))
            fillers = (
                make_qkv_steps(b + 2, *tiles[b + 2][:3]) if b + 2 < B else []
            )
        drain_tail(carry, outq, finalized)


_NC_CACHE = None


def kernel(x: np.ndarray, w_attn: np.ndarray, w_proj: np.ndarray) -> np.ndarray:
    global _NC_CACHE
    if _NC_CACHE is None:
        _NC_CACHE = build_kernel()
    nc = _NC_CACHE

    x = np.asarray(x, dtype=np.float32)
    w_attn = np.asarray(w_attn, dtype=np.float32)
    w_proj = np.asarray(w_proj, dtype=np.float32)

    xT = np.ascontiguousarray(x.reshape(BT, D).T)  # [D, BT]

    in_maps = []
    for c in range(NCORES):
        c0 = c * CW
        wq = w_attn[:, c0 : c0 + CW]
        wk = w_attn[:, D + c0 : D + c0 + CW]
        wv = w_attn[:, 2 * D + c0 : 2 * D + c0 + CW]
        wslice = np.concatenate([wq, wk, wv], axis=1)          # [D, 3*CW]
        wpacked = np.ascontiguousarray(
            wslice.reshape(KC, 128, 3 * CW).transpose(1, 0, 2)
        ).reshape(128, KC * 3 * CW)
        wpc = np.ascontiguousarray(w_proj[c0 : c0 + CW, :])    # [CW, D]
        in_maps.append({"xT": xT, "wqkv": wpacked, "wp": wpc})

    res = run_bass_kernel_spmd(nc, in_maps, core_ids=list(range(NCORES)))
    acc = np.zeros((BT, D), dtype=np.float32)
    for r in res.results:
        acc += r["out"]
    return acc.reshape(B, T, D)


if __name__ == "__main__":
    inputs = {
        "x": np.random.randn(B, T, D).astype(np.float32),
        "w_attn": (np.random.randn(D, 3 * D) / np.sqrt(D)).astype(np.float32),
        "w_proj": (np.random.randn(D, D) / np.sqrt(D)).astype(np.float32),
    }
    y = kernel(**inputs)
    print(y.shape, y.dtype)



# revision 39
# speedup vs baseline: 1.0876x; 1.0330x over previous
"""Causal self-attention on 8 NeuronCores (TRN2), tensor-parallel over heads.

Reference: y = proj(softmax(causal(Q K^T / sqrt(64))) V) with
B=4, T=2048, D=1024, H=16 heads, head_dim=64.

Sharding: each core owns 2 heads (a 128-column slice of the Q/K/V
projections and the matching 128 rows of w_proj) for all batches. Each
core emits a partial [B*T, D] output; the host sums the 8 partials
(row-parallel matmul unshard) and reshapes to [B, T, D].
"""

import sys

for _p in ("/opt/trn_rl_repo",):
    if _p not in sys.path:
        sys.path.insert(0, _p)

import numpy as np

import concourse.bass as bass
import concourse.bacc as bacc
import concourse.mybir as mybir
from concourse import tile
from concourse.bass_utils import run_bass_kernel_spmd
from concourse.masks import make_identity

B, T, D, H = 4, 2048, 1024, 16
HD = D // H           # 64 head dim
NCORES = 8
HPC = H // NCORES     # 2 heads per core
CW = HPC * HD         # 128: per-core qkv column slice width
BT = B * T            # 8192 tokens
KC = D // 128         # 8 contraction chunks for the qkv projection
NQ = 512              # query chunk
NG = NQ // 128        # 4 key-tiles per S^T group
F32 = mybir.dt.float32
F32R = mybir.dt.float32r
BF16 = mybir.dt.bfloat16
EXP = mybir.ActivationFunctionType.Exp

VST = HPC * (HD + 1)  # 130: V tile stride (per head: 64 cols + ones col)


def build_kernel():
    nc = bacc.Bacc("TRN2", target_bir_lowering=False, debug=False)

    xT = nc.dram_tensor("xT", [D, BT], F32R, kind="ExternalInput")
    # wqkv packed on host as [128, KC, 3*CW]: (kc,:) = rows kc*128..+128 of
    # [w_q_slice | w_k_slice | w_v_slice]
    wqkv = nc.dram_tensor("wqkv", [128, KC * 3 * CW], F32R, kind="ExternalInput")
    wp = nc.dram_tensor("wp", [CW, D], F32R, kind="ExternalInput")
    out = nc.dram_tensor("out", [BT, D], F32, kind="ExternalOutput")

    with tile.TileContext(nc) as tc:
        _body(tc, xT.ap(), wqkv.ap(), wp.ap(), out.ap())
    nc.compile()
    return nc


def _body(tc, xT, wqkv, wp, out):
    nc = tc.nc
    with (
        tc.tile_pool(name="const", bufs=1) as const,
        tc.tile_pool(name="xin", bufs=3) as xin,
        tc.tile_pool(name="qk", bufs=2) as qkpool,
        tc.tile_pool(name="vb", bufs=2) as vbpool,
        tc.tile_pool(name="vs", bufs=2) as vspool,
        tc.tile_pool(name="pt", bufs=28) as ptpool,
        tc.tile_pool(name="ptd", bufs=2) as ptdpool,
        tc.tile_pool(name="yt", bufs=2) as ytpool,
        tc.tile_pool(name="dn", bufs=2) as dnpool,
        tc.tile_pool(name="os", bufs=3) as ospool,
        tc.tile_pool(name="pst", bufs=2, space="PSUM") as pst,
        tc.tile_pool(name="pav", bufs=1, space="PSUM") as pav,
        tc.tile_pool(name="psm", bufs=2, space="PSUM") as psm,
    ):
        # ---- constants ----
        wq_sb = const.tile([128, KC, 3 * CW], F32R, tag="wqkv")
        nc.sync.dma_start(wq_sb[:], wqkv.rearrange("p (k c) -> p k c", k=KC))
        wp_sb = const.tile([CW, D], F32R, tag="wp")
        nc.sync.dma_start(wp_sb[:], wp[:])
        ident = const.tile([128, 128], F32, tag="ident")
        make_identity(nc, ident[:])
        ones32 = const.tile([128, (T // 128) * HPC], BF16, tag="ones32")
        nc.gpsimd.memset(ones32[:], 1.0)
        scale = 1.0 / float(np.sqrt(HD))

        def make_qkv_steps(b, qt, kt, vb):
            """Emission steps for qkv_proj(b); each step is ~1.7us of PE
            work so it can be pulled one-at-a-time as filler inside another
            batch's attention kk-stream."""
            tok0 = b * T
            xts = {}

            def load(ch):
                xt = xin.tile([128, KC, NQ], F32R, tag="xt")
                for kc in range(KC):
                    nc.sync.dma_start(
                        xt[:, kc, :],
                        xT[kc * 128 : (kc + 1) * 128,
                           tok0 + ch * NQ : tok0 + (ch + 1) * NQ],
                    )
                xts[ch] = xt

            def mtile(ch, m, dst):
                if m == 0 and ch + 1 < T // NQ:
                    load(ch + 1)
                ps = psm.tile([128, NQ], F32, tag="ps")
                for kc in range(KC):
                    nc.tensor.matmul(
                        ps[:],
                        wq_sb[:, kc, m * CW : (m + 1) * CW],
                        xts[ch][:, kc, :],
                        start=(kc == 0),
                        stop=(kc == KC - 1),
                    )
                nc.vector.tensor_copy(dst[:, ch * NQ : (ch + 1) * NQ], ps[:])

            vss = {}

            def vtile_a(ch):
                ps = psm.tile([128, NQ], F32, tag="ps")
                for kc in range(KC):
                    nc.tensor.matmul(
                        ps[:],
                        wq_sb[:, kc, 2 * CW : 3 * CW],
                        xts[ch][:, kc, :],
                        start=(kc == 0),
                        stop=(kc == KC - 1),
                    )
                vs = vspool.tile([128, NQ], F32, tag="vs")
                nc.vector.tensor_copy(vs[:], ps[:])
                vss[ch] = vs

            def vtile_b(ch):
                vs = vss[ch]
                pt2 = psm.tile([128, NQ], F32, tag="ps")
                for q in range(NG):
                    nc.tensor.transpose(
                        pt2[:, q * 128 : (q + 1) * 128],
                        vs[:, q * 128 : (q + 1) * 128],
                        ident[:],
                    )
                # pt2 holds [tok 128][tile q: h0 64 | h1 64]; scatter into vb
                # (col 0 of each 65-col head block is the ones column)
                dstv = bass.AP(
                    vb.tensor,
                    vb[:].offset + ch * NG * VST,
                    [vb[:].ap[0], [VST, NG], [HD + 1, HPC], [1, HD]],
                )
                srcv = pt2[:].rearrange("p (t h d) -> p t h d", t=NG, h=HPC)
                nc.vector.tensor_copy(dstv, srcv)

            load(0)
            # ones columns (denominator trick): col 65*j + HD of vb; written
            # up front so early AV groups never wait on the last chunk
            onesv = bass.AP(
                vb.tensor,
                vb[:].offset + HD,
                [vb[:].ap[0], [HD + 1, (T // 128) * HPC]],
            )
            nc.vector.tensor_copy(onesv, ones32[:])
            steps = []
            from functools import partial
            for ch in range(T // NQ):
                steps.append(partial(mtile, ch, 0, qt))
                steps.append(partial(mtile, ch, 1, kt))
                steps.append(partial(vtile_a, ch))
                if ch > 0:
                    # transpose+scatter of the PREVIOUS chunk: one extra step
                    # of slack between the V evacuation and the PE transpose
                    steps.append(partial(vtile_b, ch - 1))
            steps.append(partial(vtile_b, T // NQ - 1))
            return steps

        def finalize_norm(yt, jq, ytu):
            # divide O^T rows by the denominator row (broadcast to 64 parts);
            # single-op approx reciprocal (~18 bits) instead of the ~6-op
            # Newton lowering of reciprocal(). The custom-DVE recip and the
            # broadcast are partition-0-strict, so first move the denominator
            # row p64 -> p0 with a native copy.
            q0 = jq * NQ
            dn = dnpool.tile([1, HPC * NQ], F32, tag="dn")
            nc.vector.tensor_copy(dn[:], ytu[HD : HD + 1, :])
            nc.vector.reciprocal_approx_fast(dn[:], dn[:])
            dnb = dnpool.tile([HD, HPC * NQ], F32, tag="dnb")
            nc.gpsimd.partition_broadcast(dnb[:], dn[:])
            for h in range(HPC):
                nc.vector.tensor_mul(
                    yt[h * HD : (h + 1) * HD, q0 : q0 + NQ],
                    ytu[0:HD, h * NQ : (h + 1) * NQ],
                    dnb[:, h * NQ : (h + 1) * NQ],
                )

        def out_step(b, yt, tt):
            tok0 = b * T
            os_ = ospool.tile([128, D], F32, tag="os")
            for nn in range(D // NQ):
                pp = psm.tile([128, NQ], F32, tag="ps")
                nc.tensor.matmul(
                    pp[:],
                    yt[:, tt * 128 : (tt + 1) * 128],
                    wp_sb[:, nn * NQ : (nn + 1) * NQ],
                    start=True,
                    stop=True,
                )
                nc.vector.tensor_copy(os_[:, nn * NQ : (nn + 1) * NQ], pp[:])
            nc.sync.dma_start(
                out[tok0 + tt * 128 : tok0 + (tt + 1) * 128, :], os_[:]
            )

        def emit_av(avs, vb, nkk, kk, c0, ptk):
            for h in range(HPC):
                nc.tensor.matmul(
                    avs[h][0 : HD + 1, c0:NQ],
                    vb[:, kk * VST + h * (HD + 1) :
                         kk * VST + (h + 1) * (HD + 1)],
                    ptk[:, h * NQ + c0 : (h + 1) * NQ],
                    start=(kk == 0),
                    stop=(kk == nkk - 1),
                )

        def attention(b, qt, kt, vb, yt, fillers, carry, outq, finalized,
                      last=False):
            # AV matmuls run a FULL jq behind the S/exp pass: during jq's S
            # matmuls the PE drains the AVs of jq-1 (and, at jq=0, of the
            # previous batch's last jq), whose exp/affine outputs completed
            # long ago. The PE stream therefore never waits on the Act or
            # Pool engines, keeping the DVFS p-state pinned at max.
            def finish_carry():
                c = carry[0]
                if c is None or c["avq"]:
                    return
                # all AVs of the carried jq are emitted: evacuate + normalize
                avs_ = c["avs"]
                ytu = dnpool.tile([HD + 1, HPC * NQ], F32, tag="ytu")
                for h in range(HPC):
                    nc.vector.tensor_copy(
                        ytu[:, h * NQ : (h + 1) * NQ], avs_[h][0 : HD + 1, :]
                    )
                finalize_norm(c["yt"], c["jq"], ytu)
                # out tiles become eligible one jq LATER, giving the DVE
                # queue a full jq to actually execute the normalize before
                # the PE's out-proj matmul reads yt
                finalized.add((c["b"], c["jq"] - 1))
                if c["jq"] == 1:
                    finalized.add((c["b"] - 1, T // NQ - 1))
                carry[0] = None

            def emit_prev_avs(slots_left):
                c = carry[0]
                if c is None:
                    return
                n = (len(c["avq"]) + slots_left - 1) // max(slots_left, 1)
                for _ in range(n):
                    if not c["avq"]:
                        break
                    kk_, c0_, ptk_ = c["avq"].pop(0)
                    emit_av(c["avs"], c["vb"], c["nkk"], kk_, c0_, ptk_)
                finish_carry()

            def pull():
                # out-proj tiles whose normalize is emitted, else qkv steps.
                # The last batch holds back its final out tiles so drain_tail
                # has PE work to hide the last normalize chain behind.
                if (outq and (outq[0][0], outq[0][2] // NG) in finalized
                        and not (last and outq[0][2] >= 10)):
                    b_, yt_, tt_ = outq.pop(0)
                    out_step(b_, yt_, tt_)
                elif fillers:
                    fillers.pop(0)()

            for jq in range(T // NQ):
                q0 = jq * NQ
                nkk = NG * (jq + 1)
                diag0 = NG * jq
                av0 = pav.tile([128, NQ], F32, tag="av0")
                av1 = pav.tile([128, NQ], F32, tag="av1")
                cur = {"avs": [av0, av1], "nkk": nkk, "avq": [], "jq": jq,
                       "b": b, "vb": vb, "yt": yt}
                for kk in range(nkk):
                    i = kk - diag0          # >= 0 on the diagonal run
                    c0 = max(i, 0) * 128    # first valid q col in this chunk
                    w = NQ - c0
                    st = pst.tile([128, HPC * NQ], F32, tag="st")
                    for h in range(HPC):
                        nc.tensor.matmul(
                            st[:, h * NQ + c0 : (h + 1) * NQ],
                            kt[h * HD : (h + 1) * HD, kk * 128 : (kk + 1) * 128],
                            qt[h * HD : (h + 1) * HD, q0 + c0 : q0 + NQ],
                            start=True,
                            stop=True,
                        )
                    ptk = ptpool.tile([128, HPC * NQ], BF16, tag="pt")
                    stv = bass.AP(st.tensor, st[:].offset + c0,
                                  [st[:].ap[0], [NQ, HPC], [1, w]])
                    ptv = bass.AP(ptk.tensor, ptk[:].offset + c0,
                                  [ptk[:].ap[0], [NQ, HPC], [1, w]])
                    nc.scalar.activation(ptv, stv, EXP, scale=scale)
                    if i >= 0:
                        # zero q < kpart inside the 128-wide diagonal block
                        tri = bass.AP(ptk.tensor, ptk[:].offset + c0,
                                      [ptk[:].ap[0], [NQ, HPC], [1, 128]])
                        nc.gpsimd.affine_select(
                            out=tri,
                            in_=tri,
                            pattern=[[0, HPC], [1, 128]],
                            channel_multiplier=-1,
                            base=0,
                            compare_op=mybir.AluOpType.is_ge,
                            fill=0.0,
                        )
                    cur["avq"].append((kk, c0, ptk))
                    emit_prev_avs(nkk - kk)
                    if b == 0 and jq < 2:
                        pull()  # extra filler: densify the cold-start region
                    if last and carry[0] is None and len(cur["avq"]) > 1:
                        # tail batch: nothing left to interleave, so drain
                        # our own AVs one behind the S pass
                        kk_, c0_, ptk_ = cur["avq"].pop(0)
                        emit_av(cur["avs"], vb, nkk, kk_, c0_, ptk_)
                    pull()
                carry[0] = cur
            # leftover fillers (none in the steady state)
            while fillers:
                fillers.pop(0)()

        def drain_tail(carry, outq, finalized):
            # emit the final jq's AVs, its normalize, and remaining out tiles
            c = carry[0]
            if c is not None:
                while c["avq"]:
                    kk_, c0_, ptk_ = c["avq"].pop(0)
                    emit_av(c["avs"], c["vb"], c["nkk"], kk_, c0_, ptk_)
                ytu = dnpool.tile([HD + 1, HPC * NQ], F32, tag="ytu")
                for h in range(HPC):
                    nc.vector.tensor_copy(
                        ytu[:, h * NQ : (h + 1) * NQ], c["avs"][h][0 : HD + 1, :]
                    )
                finalize_norm(c["yt"], c["jq"], ytu)
                finalized.add((c["b"], c["jq"] - 1))
                finalized.add((c["b"], c["jq"]))
                carry[0] = None
            outq.sort(key=lambda e: (e[2] // NG == T // NQ - 1, e[2]))
            while outq:
                b_, yt_, tt_ = outq.pop(0)
                out_step(b_, yt_, tt_)

        tiles = []
        for b in range(B):
            qt = qkpool.tile([128, T], BF16, tag="qt")
            kt = qkpool.tile([128, T], BF16, tag="kt")
            vb = vbpool.tile([128, (T // 128) * VST], BF16, tag="vb")
            yt = ytpool.tile([128, T], F32R, tag="yt")
            tiles.append((qt, kt, vb, yt))

        # qkv(0) runs standalone; qkv(b+1) is pulled as filler during
        # attention(b); AVs lag a full jq behind their S/exp pass and cross
        # batch boundaries, so the PE pipeline never drains.
        carry = [None]
        finalized = set()
        outq = []
        steps0 = make_qkv_steps(0, *tiles[0][:3])
        for s in steps0[:4]:
            s()
        # chunks 1-3 of batch 0 ride the attention(0) filler stream: the
        # 1-filler-per-slot cadence emits chunk j just before jq=j needs it
        fillers = steps0[4:] + make_qkv_steps(1, *tiles[1][:3])
        for b in range(B):
            outq.extend((b, tiles[b][3], tt) for tt in range(T // 128))
            attention(b, *tiles[b], fillers, carry, outq, finalized,
                      last=(b == B - 1))
            fillers = (
                make_qkv_steps(b + 2, *tiles[b + 2][:3]) if b + 2 < B else []
            )
        drain_tail(carry, outq, finalized)


_NC_CACHE = None


def kernel(x: np.ndarray, w_attn: np.ndarray, w_proj: np.ndarray) -> np.ndarray:
    global _NC_CACHE
    if _NC_CACHE is None:
        _NC_CACHE = build_kernel()
    nc = _NC_CACHE

    x = np.asarray(x, dtype=np.float32)
    w_attn = np.asarray(w_attn, dtype=np.float32)
    w_proj = np.asarray(w_proj, dtype=np.float32)

    xT = np.ascontiguousarray(x.reshape(BT, D).T)  # [D, BT]

    in_maps = []
    for c in range(NCORES):
        c0 = c * CW
        wq = w_attn[:, c0 : c0 + CW]
        wk = w_attn[:, D + c0 : D + c0 + CW]
        wv = w_attn[:, 2 * D + c0 : 2 * D + c0 + CW]
        wslice = np.concatenate([wq, wk, wv], axis=1)          # [D, 3*CW]
        wpacked = np.ascontiguousarray(
            wslice.reshape(KC, 128, 3 * CW).transpose(1, 0, 2)
        ).reshape(128, KC * 3 * CW)
        wpc = np.ascontiguousarray(w_proj[c0 : c0 + CW, :])    # [CW, D]
        in_maps.append({"xT": xT, "wqkv": wpacked, "wp": wpc})

    res = run_bass_kernel_spmd(nc, in_maps, core_ids=list(range(NCORES)))
    acc = np.zeros((BT, D), dtype=np.float32)
    for r in res.results:
        acc += r["out"]
    return acc.reshape(B, T, D)


if __name__ == "__main__":
    inputs = {
        "x": np.random.randn(B, T, D).astype(np.float32),
        "w_attn": (np.random.randn(D, 3 * D) / np.sqrt(D)).astype(np.float32),
        "w_proj": (np.random.randn(D, D) / np.sqrt(D)).astype(np.float32),
    }
    y = kernel(**inputs)
    print(y.shape, y.dtype)

